# revision 1
# baseline (speedup 1.0000x reference)
"""NT-Xent contrastive loss on 8 Trainium2 NeuronCores (Bass/Tile).

Strategy (no collectives -- measured ncfw latency floor ~85us makes the
all-gather hint design strictly worse):
  * Host pre-transposes embedded_data to embT [2048, 8192] (pure layout).
  * Slab cover: core c loads the 4 row-slabs S_c = {c, c+1, c+2, c+4} (mod 8)
    of emb (32 MiB/core). Every slab PAIR meets on some core (Z8 difference
    cover: slot-pairs at differences 1,2,3,4), so each of the 36 distinct
    1024x1024 blocks of the 8192x8192 similarity matrix is computed once
    globally; block (i,j) yields exp-row-sums for slab i (ACT accum) AND
    exp-col-sums for slab j (ones-matmul), exploiting sim symmetry.
  * Per core, uniform SPMD program: head matmul out_headT = W.T @ embT_slab
    (fp32r, 1 cyc/row), L2 normalize via ones-matmul normsq + Sqrt +
    reciprocal + K=1 broadcast matmul, then 5 sim blocks (diag + 4 pairs):
    psum [128,1024] fp32 -> ACT exp(10*x) with fused row-sum accum ->
    f32r exp tile -> ones-matmul col-sums. Diagonal exp values extracted
    exactly via a shifted-identity mask (mult+reduce) and subtracted on host.
  * pos term: elementwise product of slabs c and c+4 + ones-matmul -> the
    positive-pair similarities; log(pos) = 10*possim exactly (no exp needed).
  * Host (fp64): sums partial row/col contributions, subtracts diag,
    loss = -mean(10*possim - log(neg)).
"""
import numpy as np

SLOTS = [(c, (c + 1) % 8, (c + 2) % 8, (c + 4) % 8) for c in range(8)]
# blocks in local slot coords: (stationary, moving). B0 = diag.
BLOCKS = [(0, 0), (0, 1), (0, 2), (1, 3), (0, 3)]

_CACHE = {}


def _build():
    if "nc" in _CACHE:
        return _CACHE["nc"]
    import concourse.bacc as bacc
    import concourse.tile as tile
    import concourse.mybir as mybir

    F32, F32R = mybir.dt.float32, mybir.dt.float32r
    AF = mybir.ActivationFunctionType
    ALU = mybir.AluOpType

    nc = bacc.Bacc("TRN2", num_devices=8, debug=False)
    a_emb = nc.dram_tensor("embT", [2048, 4096], F32, kind="ExternalInput").ap()
    a_W = nc.dram_tensor("W", [2048, 256], F32, kind="ExternalInput").ap()
    a_b = nc.dram_tensor("b", [256], F32, kind="ExternalInput").ap()
    a_ones = nc.dram_tensor("ones", [128, 128], F32, kind="ExternalInput").ap()
    a_mask = nc.dram_tensor("mask", [128, 2048], F32, kind="ExternalInput").ap()
    o_rp = nc.dram_tensor("rowpart", [5, 1024], F32, kind="ExternalOutput").ap()
    o_cp = nc.dram_tensor("colpart", [4, 1024], F32, kind="ExternalOutput").ap()
    o_dg = nc.dram_tensor("diagexp", [1, 1024], F32, kind="ExternalOutput").ap()
    o_ps = nc.dram_tensor("possim", [1, 1024], F32, kind="ExternalOutput").ap()

    with tile.TileContext(nc) as tc:
        with tc.tile_pool(name="sb", bufs=1) as sb, \
             tc.tile_pool(name="emb", bufs=10) as embp, \
             tc.tile_pool(name="work", bufs=2) as wk, \
             tc.tile_pool(name="expp", bufs=3) as expp, \
             tc.tile_pool(name="headp", bufs=1, space="PSUM") as headp, \
             tc.tile_pool(name="simp", bufs=2, space="PSUM") as simp, \
             tc.tile_pool(name="csp", bufs=2, space="PSUM") as csp:

            t_W = sb.tile([128, 16, 256], F32R, name="t_W")
            nc.sync.dma_start(t_W[:], a_W.bitcast(F32R).rearrange("(kc p) d -> p kc d", p=128))
            t_b = sb.tile([128, 2], F32, name="t_b")
            nc.sync.dma_start(t_b[:], a_b.rearrange("(dh p) -> p dh", p=128))
            ones_col = sb.tile([128, 1], F32R, name="ones_col")
            nc.sync.dma_start(ones_col[:], a_ones.bitcast(F32R)[:, 0:1])
            ones_row = sb.tile([1, 128], F32, name="ones_row")
            nc.sync.dma_start(ones_row[:], a_ones[0:1, :])
            t_mask = sb.tile([128, 2048], F32, name="t_mask")
            nc.sync.dma_start(t_mask[:], a_mask[:])

            # staging accumulators
            rp_st = sb.tile([128, 5, 8], F32, name="rp_st")
            dg_st = sb.tile([128, 8], F32, name="dg_st")
            cp_st = sb.tile([1, 4096], F32, name="cp_st")
            ps_st = sb.tile([1, 1024], F32, name="ps_st")

            t_on = [sb.tile([128, 2, 1024], F32R, name=f"t_on{k}") for k in range(4)]

            def stage_a(k):
                t_h = wk.tile([128, 2, 1024], F32, name="t_h", tag="th")
                for h in range(2):
                    tes = []
                    for g in range(8):
                        t_e = embp.tile([128, 2, 512], F32R, name="t_e", tag="emb")
                        src = a_emb.bitcast(F32R)[256 * g:256 * (g + 1),
                                                  1024 * k + 512 * h:1024 * k + 512 * (h + 1)]
                        nc.sync.dma_start(t_e[:], src.rearrange("(c p) r -> p c r", p=128))
                        tes.append(t_e)
                    p_h = headp.tile([128, 2, 512], F32, name="p_h", tag="head")
                    for g in range(8):
                        for cc in range(2):
                            kk = 2 * g + cc
                            for dh in range(2):
                                nc.tensor.matmul(
                                    p_h[:, dh, :],
                                    t_W[:, kk, dh * 128:(dh + 1) * 128],
                                    tes[g][:, cc, :],
                                    start=(kk == 0), stop=(kk == 15),
                                )
                    for dh in range(2):
                        nc.vector.tensor_scalar_add(
                            t_h[:, dh, 512 * h:512 * (h + 1)], p_h[:, dh, :],
                            t_b[:, dh:dh + 1])
                t_sq = wk.tile([128, 2, 1024], F32R, name="t_sq", tag="sq")
                nc.vector.tensor_tensor(t_sq[:], t_h[:], t_h[:], ALU.mult)
                p_ns = [csp.tile([1, 512], F32, name=f"p_ns{nb}", tag="cs") for nb in range(2)]
                for nb in range(2):
                    for dh in range(2):
                        nc.tensor.matmul(p_ns[nb][:], ones_col[:],
                                         t_sq[:, dh, 512 * nb:512 * (nb + 1)],
                                         start=(dh == 0), stop=(dh == 1))
                t_nrm = wk.tile([1, 1024], F32, name="t_nrm", tag="nrm")
                for nb in range(2):
                    nc.scalar.activation(t_nrm[:, 512 * nb:512 * (nb + 1)], p_ns[nb][:], AF.Sqrt)
                t_ri = wk.tile([1, 1024], F32, name="t_ri", tag="ri")
                nc.vector.reciprocal(t_ri[:], t_nrm[:])
                p_bc = headp.tile([128, 2, 512], F32, name="p_bc", tag="head")
                for nb in range(2):
                    nc.tensor.matmul(p_bc[:, nb, :], ones_row[:],
                                     t_ri[:, 512 * nb:512 * (nb + 1)], start=True, stop=True)
                bc_flat = p_bc[:].rearrange("p a b -> p (a b)")
                for dh in range(2):
                    nc.vector.tensor_tensor(t_on[k][:, dh, :], t_h[:, dh, :], bc_flat, ALU.mult)

            def block(bslot, a, bm):
                p_cs = None
                if bslot > 0:
                    p_cs = [csp.tile([1, 512], F32, name=f"p_cs{bslot}_{nb}", tag="cs")
                            for nb in range(2)]
                for mb in range(8):
                    p_sim = simp.tile([128, 1024], F32, name="p_sim", tag="sim")
                    for dh in range(2):
                        for nb in range(2):
                            nc.tensor.matmul(
                                p_sim[:, 512 * nb:512 * (nb + 1)],
                                t_on[a][:, dh, 128 * mb:128 * (mb + 1)],
                                t_on[bm][:, dh, 512 * nb:512 * (nb + 1)],
                                start=(dh == 0), stop=(dh == 1))
                    t_exp = expp.tile([128, 1024], F32R, name="t_exp", tag="exp")
                    nc.scalar.activation(t_exp[:], p_sim[:], AF.Exp, scale=10.0,
                                         accum_out=rp_st[:, bslot, mb:mb + 1])
                    if bslot > 0:
                        for nb in range(2):
                            nc.tensor.matmul(p_cs[nb][:], ones_col[:],
                                             t_exp[:, 512 * nb:512 * (nb + 1)],
                                             start=(mb == 0), stop=(mb == 7))
                    else:
                        t_sc = expp.tile([128, 1024], F32, name="t_sc", tag="sc")
                        nc.vector.tensor_tensor(
                            t_sc[:], t_exp[:].bitcast(F32),
                            t_mask[:, 1024 - 128 * mb:2048 - 128 * mb], ALU.mult)
                        nc.vector.tensor_reduce(dg_st[:, mb:mb + 1], t_sc[:],
                                                mybir.AxisListType.X, ALU.add)
                if bslot > 0:
                    for nb in range(2):
                        nc.vector.tensor_copy(
                            cp_st[0:1, 1024 * (bslot - 1) + 512 * nb:
                                  1024 * (bslot - 1) + 512 * (nb + 1)], p_cs[nb][:])

            stage_a(0)
            block(0, 0, 0)
            stage_a(1)
            block(1, 0, 1)
            stage_a(2)
            block(2, 0, 2)
            stage_a(3)
            block(3, 1, 3)
            block(4, 0, 3)

            # pos: elementwise product slabs slot0 x slot3, column sums over d
            t_pp = wk.tile([128, 2, 1024], F32R, name="t_pp", tag="sq")
            for dh in range(2):
                nc.vector.tensor_tensor(t_pp[:, dh, :], t_on[0][:, dh, :].bitcast(F32),
                                        t_on[3][:, dh, :].bitcast(F32), ALU.mult)
            p_ps = [csp.tile([1, 512], F32, name=f"p_ps{nb}", tag="cs") for nb in range(2)]
            for nb in range(2):
                for dh in range(2):
                    nc.tensor.matmul(p_ps[nb][:], ones_col[:],
                                     t_pp[:, dh, 512 * nb:512 * (nb + 1)],
                                     start=(dh == 0), stop=(dh == 1))
                nc.vector.tensor_copy(ps_st[0:1, 512 * nb:512 * (nb + 1)], p_ps[nb][:])

            # final DMAs
            for bslot in range(5):
                nc.sync.dma_start(
                    o_rp[bslot:bslot + 1, :].rearrange("o (m p) -> p (o m)", p=128),
                    rp_st[:, bslot, :])
            nc.sync.dma_start(o_dg.rearrange("o (m p) -> p (o m)", p=128), dg_st[:])
            nc.sync.dma_start(o_cp.rearrange("a r -> (a r)")[None, :], cp_st[:])
            nc.sync.dma_start(o_ps[:], ps_st[:])

    nc.compile()
    _CACHE["nc"] = nc
    return nc


def _host_inputs(embedded_data, W, b):
    embT = np.ascontiguousarray(np.asarray(embedded_data, dtype=np.float32).T)
    W = np.asarray(W, dtype=np.float32)
    b = np.asarray(b, dtype=np.float32)
    mask = np.zeros((128, 2048), np.float32)
    mask[np.arange(128), np.arange(128) + 1024] = 1.0
    ones = np.ones((128, 128), np.float32)
    in_maps = []
    for c in range(8):
        cols = np.concatenate(
            [embT[:, 1024 * s:1024 * (s + 1)] for s in SLOTS[c]], axis=1)
        in_maps.append({"embT": np.ascontiguousarray(cols), "W": W, "b": b,
                        "ones": ones, "mask": mask})
    return in_maps


def _combine(results):
    neg = np.zeros(8192, np.float64)
    pos = np.zeros(8192, np.float64)
    for c in range(8):
        S = SLOTS[c]
        rp = results[c]["rowpart"].astype(np.float64)
        cp = results[c]["colpart"].astype(np.float64)
        dg = results[c]["diagexp"].astype(np.float64).ravel()
        sl = [np.s_[1024 * s:1024 * (s + 1)] for s in S]
        neg[sl[0]] += rp[0] - dg          # diag block, self-sim removed
        neg[sl[0]] += rp[1]; neg[sl[1]] += cp[0]   # B1 (0,1)
        neg[sl[0]] += rp[2]; neg[sl[2]] += cp[1]   # B2 (0,2)
        neg[sl[1]] += rp[3]; neg[sl[3]] += cp[2]   # B3 (1,3)
        if c < 4:                                   # B4 (0,3) dedup: cores 0-3
            neg[sl[0]] += rp[4]; neg[sl[3]] += cp[3]
            ps = results[c]["possim"].astype(np.float64).ravel()
            pos[sl[0]] = ps
            pos[1024 * S[3]:1024 * (S[3] + 1)] = ps
    loss = -np.mean(10.0 * pos - np.log(neg))
    return np.float32(loss)


def run(embedded_data, W, b, trace=False):
    from concourse import bass_utils
    nc = _build()
    in_maps = _host_inputs(embedded_data, W, b)
    res = bass_utils.run_bass_kernel_spmd(nc, in_maps, core_ids=list(range(8)),
                                          trace=trace)
    return _combine(res.results), res


def kernel(embedded_data, W, b):
    loss, _ = run(embedded_data, W, b, trace=False)
    return np.asarray(loss, dtype=np.float32)



# revision 14
# speedup vs baseline: 1.9616x; 1.9616x over previous
"""NT-Xent contrastive loss on 8 Trainium2 NeuronCores (Bass/Tile), fp8.

Strategy (no collectives; ncfw collective latency floor ~85us):
  * Host casts embT to fp8e4 [2048, 8192] (sigma=1 fits e4m3) and W*64 to
    fp8e4; b*64 stays f32.  Slab cover: core c loads the 4 column-slabs
    S_c = {c, c+1, c+2, c+4} (mod 8) of embT (8.4 MB/core).  Every slab
    pair meets on some core (Z8 difference cover), so each distinct
    1024x1024 block of the 8192x8192 similarity matrix is computed once
    globally (the diff-4 block is deduped on host: cores 0-3 win).
  * Per core: head matmul in fp8 DoubleRow (K=256/instr, 0.5 cyc/row)
    -> h' = 64h in psum -> bias-add copy to bf16 (Pool dh0 / DVE dh1).
    L2 norm: nsq via bf16 ones-matmul into a [33,512] psum tile (rows 0
    and 32), then r = exp(-0.5*ln(nsq) + ln8) on ACT (ln+exp share one
    activation table with the sim exp => zero table reloads), broadcast
    down partitions with gpsimd partition_broadcast, t_on = h*r in fp8e4
    (= 8 * normalized out).
  * 5 sim blocks/core (diag + 4 pairs): one DoubleRow matmul per
    [128,1024] psum tile; diag killed pre-exp with an additive -1e9
    shifted mask (DVE); ACT exp(0.15625*x) with fused row-sum accum
    writes fp8e5 exp values; column sums via DoubleRow ones-matmul over
    mb-pair-interleaved e5 tiles at the end.
  * pos: bf16 product of t_h slabs 0,3 + ones-matmul + r-scales; host
    divides by 64.  Host combine in fp64.
"""
import math
import numpy as np
import ml_dtypes

SLOTS = [(c, (c + 1) % 8, (c + 2) % 8, (c + 4) % 8) for c in range(8)]
# sim units: (stationary slot, moving slot, e5 colsum slot or None)
UNITS = [(0, 0, None), (0, 1, 0), (0, 2, 1), (0, 3, 2), (1, 3, 3)]
LN8 = math.log(8.0)

_CACHE = {}


def _build():
    if "nc" in _CACHE:
        return _CACHE["nc"]
    import concourse.bacc as bacc
    import concourse.tile as tile
    import concourse.mybir as mybir

    F32 = mybir.dt.float32
    BF16 = mybir.dt.bfloat16
    E4 = mybir.dt.float8e4
    E5 = mybir.dt.float8e5
    AF = mybir.ActivationFunctionType
    ALU = mybir.AluOpType
    DR = mybir.MatmulPerfMode.DoubleRow

    nc = bacc.Bacc("TRN2", num_devices=8, debug=False)
    a_emb = nc.dram_tensor("embT8", [2048, 4096], E4, kind="ExternalInput").ap()
    a_W = nc.dram_tensor("W8", [2048, 256], E4, kind="ExternalInput").ap()
    a_b = nc.dram_tensor("b64", [256], F32, kind="ExternalInput").ap()
    a_o1 = nc.dram_tensor("onesbf", [128, 1], BF16, kind="ExternalInput").ap()
    a_o5 = nc.dram_tensor("ones5", [128, 256], E5, kind="ExternalInput").ap()
    a_mask = nc.dram_tensor("mask", [128, 2048], F32, kind="ExternalInput").ap()
    o_rp = nc.dram_tensor("rowpart", [128, 40], F32, kind="ExternalOutput").ap()
    o_cp = nc.dram_tensor("colpart", [1, 4096], F32, kind="ExternalOutput").ap()
    o_ps = nc.dram_tensor("possim", [1, 1024], F32, kind="ExternalOutput").ap()

    with tile.TileContext(nc) as tc:
        with tc.tile_pool(name="sb", bufs=1) as sb, \
             tc.tile_pool(name="wk", bufs=2) as wk, \
             tc.tile_pool(name="hp", bufs=1, space="PSUM") as hp, \
             tc.tile_pool(name="simp", bufs=2, space="PSUM") as simp, \
             tc.tile_pool(name="smp", bufs=2, space="PSUM") as smp:

            # ---- persistent tiles + prologue DMAs
            t_W = sb.tile([128, 8, 2, 2, 128], E4, name="t_W")
            nc.sync.dma_start(
                t_W[:],
                a_W.rearrange("(kk j p) (dh f) -> p kk j dh f",
                              kk=8, j=2, p=128, dh=2, f=128))
            t_b = sb.tile([128, 2], F32, name="t_b")
            nc.sync.dma_start(t_b[:], a_b.rearrange("(dh p) -> p dh", p=128))
            t_o1 = sb.tile([128, 1], BF16, name="t_o1")
            nc.sync.dma_start(t_o1[:], a_o1[:])
            t_o5 = sb.tile([128, 2, 128], E5, name="t_o5")
            nc.sync.dma_start(t_o5[:], a_o5.rearrange("p (j f) -> p j f",
                                                      j=2, f=128))
            t_mask = sb.tile([128, 2048], F32, name="t_mask")
            nc.sync.dma_start(t_mask[:], a_mask[:])

            t_e8 = []
            for k in range(4):
                row = []
                for kk in range(8):
                    t = sb.tile([128, 2, 1024], E4, name=f"t_e8_{k}_{kk}")
                    src = a_emb[256 * kk:256 * (kk + 1),
                                1024 * k:1024 * (k + 1)]
                    nc.sync.dma_start(t[:], src.rearrange("(j p) s -> p j s",
                                                          j=2, p=128))
                    row.append(t)
                t_e8.append(row)

            t_h = [sb.tile([128, 2, 1024], BF16, name=f"t_h{k}")
                   for k in range(4)]
            t_r = [[sb.tile([1, 512], F32, name=f"t_r{k}_{nh}")
                    for nh in range(2)] for k in range(4)]
            t_on = sb.tile([128, 2, 4, 1024], E4, name="t_on")
            t_e5 = sb.tile([128, 8, 4, 1024], E5, name="t_e5")
            t_scr = sb.tile([128, 1024], E5, name="t_scr")
            rp_st = sb.tile([128, 40], F32, name="rp_st")
            cp_st = sb.tile([1, 4096], F32, name="cp_st")
            ps_st = sb.tile([1, 1024], F32, name="ps_st")

            def stage(k):
                for dh in range(2):
                    H = hp.tile([128, 1024], F32, name=f"H{k}_{dh}", tag="H")
                    for kk in range(8):
                        for h in range(2):
                            nc.tensor.matmul(
                                H[:, 512 * h:512 * (h + 1)],
                                t_W[:, kk, :, dh, :],
                                t_e8[k][kk][:, :, 512 * h:512 * (h + 1)],
                                start=(kk == 0), stop=(kk == 7),
                                perf_mode=DR)
                    nc.vector.tensor_scalar_add(t_h[k][:, dh, :], H[:],
                                                t_b[:, dh:dh + 1])
                t_sq = wk.tile([128, 2, 1024], BF16, name="t_sq", tag="sq")
                nc.vector.tensor_tensor(t_sq[:], t_h[k][:], t_h[k][:],
                                        ALU.mult)
                # t_o1 holds 1/64, so nsq psum = nsq'/64 and
                # exp(-0.5*ln(x)) = 8/sqrt(nsq') -- no activation bias needed
                r_bc = wk.tile([128, 1024], F32, name="r_bc", tag="rbc")
                for nh in range(2):
                    nsq = smp.tile([1, 512], F32, name=f"nsq{k}_{nh}",
                                   tag="sm")
                    for dh in range(2):
                        nc.tensor.matmul(
                            nsq[:], t_o1[:],
                            t_sq[:, dh, 512 * nh:512 * (nh + 1)],
                            start=(dh == 0), stop=(dh == 1))
                    nln = wk.tile([1, 512], F32, name="nln", tag="nln")
                    nc.scalar.activation(nln[:], nsq[:], AF.Ln)
                    nc.scalar.activation(t_r[k][nh][:], nln[:], AF.Exp,
                                         scale=-0.5)
                    nc.gpsimd.partition_broadcast(
                        r_bc[:, 512 * nh:512 * (nh + 1)], t_r[k][nh][:])
                for dh in range(2):
                    nc.vector.tensor_tensor(t_on[:, dh, k, :],
                                            t_h[k][:, dh, :], r_bc[:],
                                            ALU.mult)

            def unit(u, a, b, e5slot, mb):
                ps = simp.tile([128, 1024], F32, name="ps", tag="ps")
                for nb in range(2):
                    nc.tensor.matmul(ps[:, 512 * nb:512 * (nb + 1)],
                                     t_on[:, :, a, 128 * mb:128 * (mb + 1)],
                                     t_on[:, :, b, 512 * nb:512 * (nb + 1)],
                                     start=True, stop=True, perf_mode=DR)
                if a == b:
                    nc.vector.tensor_tensor(
                        ps[:], ps[:],
                        t_mask[:, 1024 - 128 * mb:2048 - 128 * mb], ALU.add)
                dest = t_scr[:] if e5slot is None else t_e5[:, mb, e5slot, :]
                nc.scalar.activation(dest, ps[:], AF.Exp, scale=0.15625,
                                     accum_out=rp_st[:, u * 8 + mb:
                                                     u * 8 + mb + 1])

            stage(0)
            for mb in range(8):
                unit(0, *UNITS[0][:2], UNITS[0][2], mb)
            stage(1)
            for mb in range(8):
                unit(1, *UNITS[1][:2], UNITS[1][2], mb)
            stage(2)
            for mb in range(8):
                unit(2, *UNITS[2][:2], UNITS[2][2], mb)
            stage(3)
            for mb in range(8):
                unit(3, *UNITS[3][:2], UNITS[3][2], mb)
            for mb in range(8):
                unit(4, *UNITS[4][:2], UNITS[4][2], mb)

            # pos: bf16 product of t_h slabs 0 and 3, ones-matmul, r-scales
            t_pp = wk.tile([128, 2, 1024], BF16, name="t_pp", tag="sq")
            nc.vector.tensor_tensor(t_pp[:], t_h[0][:], t_h[3][:], ALU.mult)
            for nh in range(2):
                pr = smp.tile([1, 512], F32, name=f"rawdot{nh}", tag="sm")
                for dh in range(2):
                    nc.tensor.matmul(pr[:], t_o1[:],
                                     t_pp[:, dh, 512 * nh:512 * (nh + 1)],
                                     start=(dh == 0), stop=(dh == 1))
                tmp = wk.tile([1, 512], F32, name=f"ptmp{nh}", tag="nln")
                nc.vector.tensor_tensor(tmp[:], pr[:], t_r[0][nh][:],
                                        ALU.mult)
                nc.vector.tensor_tensor(
                    ps_st[0:1, 512 * nh:512 * (nh + 1)], tmp[:],
                    t_r[3][nh][:], ALU.mult)

            # colsums from e5 exp tiles (DoubleRow over mb pairs)
            for ci in range(4):
                for nh in range(2):
                    cs = smp.tile([128, 512], F32, name=f"cs{ci}_{nh}",
                                  tag="sm")
                    for jj in range(4):
                        nc.tensor.matmul(
                            cs[:], t_o5[:],
                            t_e5[:, 2 * jj:2 * jj + 2, ci,
                                 512 * nh:512 * (nh + 1)],
                            start=(jj == 0), stop=(jj == 3), perf_mode=DR)
                    nc.vector.tensor_copy(
                        cp_st[0:1, 1024 * ci + 512 * nh:
                              1024 * ci + 512 * (nh + 1)], cs[0:1, :])

            nc.sync.dma_start(o_rp, rp_st[:])
            nc.sync.dma_start(o_cp, cp_st[:])
            nc.sync.dma_start(o_ps, ps_st[:])

    nc.compile()
    _CACHE["nc"] = nc
    return nc


def _host_inputs(embedded_data, W, b):
    E4np = ml_dtypes.float8_e4m3
    E5np = ml_dtypes.float8_e5m2
    emb = np.asarray(embedded_data, dtype=np.float32)
    embT8 = np.ascontiguousarray(emb.T).astype(E4np)      # [2048, 8192]
    W8 = (np.asarray(W, dtype=np.float32) * 64.0).astype(E4np)
    b64 = (np.asarray(b, dtype=np.float32) * 64.0).astype(np.float32)
    o1 = np.full((128, 1), 1.0 / 64.0, ml_dtypes.bfloat16)
    o5 = np.ones((128, 256), E5np)
    mask = np.zeros((128, 2048), np.float32)
    mask[np.arange(128), np.arange(128) + 1024] = -1e9
    in_maps = []
    for c in range(8):
        cols = np.concatenate(
            [embT8[:, 1024 * s:1024 * (s + 1)] for s in SLOTS[c]], axis=1)
        in_maps.append({"embT8": np.ascontiguousarray(cols), "W8": W8,
                        "b64": b64, "onesbf": o1, "ones5": o5, "mask": mask})
    return in_maps


def _combine(results):
    neg = np.zeros(8192, np.float64)
    pos = np.zeros(8192, np.float64)
    for c in range(8):
        S = SLOTS[c]
        rp = results[c]["rowpart"].astype(np.float64)     # [128, 40]
        cp = results[c]["colpart"].astype(np.float64).ravel()
        ps = results[c]["possim"].astype(np.float64)
        sl = [np.s_[1024 * s:1024 * (s + 1)] for s in S]
        for u, (astat, _, _) in enumerate(UNITS):
            if u == 3 and c >= 4:
                continue                                   # diff-4 dedup
            dst = 1024 * S[astat]
            for mb in range(8):
                neg[dst + 128 * mb:dst + 128 * (mb + 1)] += rp[:, 8 * u + mb]
        neg[sl[1]] += cp[0:1024]
        neg[sl[2]] += cp[1024:2048]
        if c < 4:
            neg[sl[3]] += cp[2048:3072]
        neg[sl[3]] += cp[3072:4096]
        if c < 4:
            possim = ps.ravel()
            pos[sl[0]] = possim
            pos[sl[3]] = possim
    loss = -np.mean(10.0 * pos - np.log(neg))
    return np.float32(loss)


def run(embedded_data, W, b, trace=False):
    from concourse import bass_utils
    nc = _build()
    in_maps = _host_inputs(embedded_data, W, b)
    res = bass_utils.run_bass_kernel_spmd(nc, in_maps, core_ids=list(range(8)),
                                          trace=trace)
    return _combine(res.results), res


def kernel(embedded_data, W, b):
    loss, _ = run(embedded_data, W, b, trace=False)
    return np.asarray(loss, dtype=np.float32)


# revision 16
# speedup vs baseline: 2.0973x; 1.0692x over previous
"""NT-Xent contrastive loss on 8 Trainium2 NeuronCores (Bass/Tile), fp8.

Strategy (no collectives; ncfw collective latency floor ~85us):
  * Host casts embT to fp8e4 [2048, 8192] (sigma=1 fits e4m3) and W*64 to
    fp8e4; b*64 stays f32.  Slab cover: core c loads the 4 column-slabs
    S_c = {c, c+1, c+2, c+4} (mod 8) of embT (8.4 MB/core).  Every slab
    pair meets on some core (Z8 difference cover), so each distinct
    1024x1024 block of the 8192x8192 similarity matrix is computed once
    globally (the diff-4 block is deduped on host: cores 0-3 win).
  * Per core: head matmul in fp8 DoubleRow (K=256/instr, 0.5 cyc/row)
    -> h' = 64h in psum -> bias-add copy to bf16 (Pool dh0 / DVE dh1).
    L2 norm: nsq via bf16 ones-matmul into a [33,512] psum tile (rows 0
    and 32), then r = exp(-0.5*ln(nsq) + ln8) on ACT (ln+exp share one
    activation table with the sim exp => zero table reloads), broadcast
    down partitions with gpsimd partition_broadcast, t_on = h*r in fp8e4
    (= 8 * normalized out).
  * 5 sim blocks/core (diag + 4 pairs): one DoubleRow matmul per
    [128,1024] psum tile; diag killed pre-exp with an additive -1e9
    shifted mask (DVE); ACT exp(0.15625*x) with fused row-sum accum
    writes fp8e5 exp values; column sums via DoubleRow ones-matmul over
    mb-pair-interleaved e5 tiles at the end.
  * pos: bf16 product of t_h slabs 0,3 + ones-matmul + r-scales; host
    divides by 64.  Host combine in fp64.
"""
import math
import numpy as np
import ml_dtypes

SLOTS = [(c, (c + 1) % 8, (c + 2) % 8, (c + 4) % 8) for c in range(8)]
# sim units: (stationary slot, moving slot, e5 colsum slot or None)
UNITS = [(0, 0, None), (0, 1, 0), (0, 2, 1), (0, 3, 2), (1, 3, 3)]
LN8 = math.log(8.0)

_CACHE = {}


def _build():
    if "nc" in _CACHE:
        return _CACHE["nc"]
    import concourse.bacc as bacc
    import concourse.tile as tile
    import concourse.mybir as mybir

    F32 = mybir.dt.float32
    BF16 = mybir.dt.bfloat16
    E4 = mybir.dt.float8e4
    E5 = mybir.dt.float8e5
    AF = mybir.ActivationFunctionType
    ALU = mybir.AluOpType
    DR = mybir.MatmulPerfMode.DoubleRow

    nc = bacc.Bacc("TRN2", num_devices=8, debug=False)
    a_emb = nc.dram_tensor("embT8", [2048, 4096], E4, kind="ExternalInput").ap()
    a_W = nc.dram_tensor("W8", [2048, 256], E4, kind="ExternalInput").ap()
    a_b = nc.dram_tensor("b64", [256], F32, kind="ExternalInput").ap()
    a_o1 = nc.dram_tensor("onesbf", [128, 1], BF16, kind="ExternalInput").ap()
    a_o5 = nc.dram_tensor("ones5", [128, 256], E5, kind="ExternalInput").ap()
    a_mask = nc.dram_tensor("mask", [128, 2048], F32, kind="ExternalInput").ap()
    o_rp = nc.dram_tensor("rowpart", [128, 40], F32, kind="ExternalOutput").ap()
    o_cp = nc.dram_tensor("colpart", [1, 4096], F32, kind="ExternalOutput").ap()
    o_ps = nc.dram_tensor("possim", [1, 1024], F32, kind="ExternalOutput").ap()

    with tile.TileContext(nc) as tc:
        with tc.tile_pool(name="sb", bufs=1) as sb, \
             tc.tile_pool(name="wk", bufs=2) as wk, \
             tc.tile_pool(name="hp", bufs=2, space="PSUM") as hp, \
             tc.tile_pool(name="simp", bufs=2, space="PSUM") as simp, \
             tc.tile_pool(name="smp", bufs=1, space="PSUM") as smp:

            # ---- persistent tiles + prologue DMAs
            t_W = sb.tile([128, 8, 2, 2, 128], E4, name="t_W")
            nc.sync.dma_start(
                t_W[:],
                a_W.rearrange("(kk j p) (dh f) -> p kk j dh f",
                              kk=8, j=2, p=128, dh=2, f=128))
            t_b = sb.tile([128, 2], F32, name="t_b")
            nc.sync.dma_start(t_b[:], a_b.rearrange("(dh p) -> p dh", p=128))
            t_o1 = sb.tile([128, 1], BF16, name="t_o1")
            nc.sync.dma_start(t_o1[:], a_o1[:])
            t_o5 = sb.tile([128, 2, 128], E5, name="t_o5")
            nc.sync.dma_start(t_o5[:], a_o5.rearrange("p (j f) -> p j f",
                                                      j=2, f=128))
            t_mask = sb.tile([128, 2048], F32, name="t_mask")
            nc.sync.dma_start(t_mask[:], a_mask[:])

            t_e8 = []
            for k in range(4):
                row = []
                for kk in range(8):
                    t = sb.tile([128, 2, 1024], E4, name=f"t_e8_{k}_{kk}")
                    src = a_emb[256 * kk:256 * (kk + 1),
                                1024 * k:1024 * (k + 1)]
                    nc.sync.dma_start(t[:], src.rearrange("(j p) s -> p j s",
                                                          j=2, p=128))
                    row.append(t)
                t_e8.append(row)

            t_h = [sb.tile([128, 2, 1024], BF16, name=f"t_h{k}")
                   for k in range(4)]
            t_r_tiles = [None] * 4
            t_on = sb.tile([128, 2, 4, 1024], E4, name="t_on")
            t_e5 = sb.tile([128, 8, 4, 1024], E5, name="t_e5")
            t_scr = sb.tile([128, 1024], E5, name="t_scr")
            rp_st = sb.tile([128, 40], F32, name="rp_st")
            cp_st = sb.tile([1, 4096], F32, name="cp_st")
            ps_st = sb.tile([1, 1024], F32, name="ps_st")

            def head_chain(k, dh, h):
                H = hp.tile([128, 512], F32, name=f"H{k}_{dh}_{h}", tag="H")
                for kk in range(8):
                    nc.tensor.matmul(
                        H[:], t_W[:, kk, :, dh, :],
                        t_e8[k][kk][:, :, 512 * h:512 * (h + 1)],
                        start=(kk == 0), stop=(kk == 7), perf_mode=DR)
                nc.vector.tensor_scalar_add(
                    t_h[k][:, dh, 512 * h:512 * (h + 1)], H[:],
                    t_b[:, dh:dh + 1])

            def norm(k):
                t_sq = wk.tile([128, 2, 1024], BF16, name="t_sq", tag="sq")
                nc.vector.tensor_tensor(t_sq[:], t_h[k][:], t_h[k][:],
                                        ALU.mult)
                # t_o1 holds 1/64, so nsq psum = nsq'/64 and
                # exp(-0.5*ln(x)) = 8/sqrt(nsq') -- no activation bias needed
                r_bc = wk.tile([128, 1024], F32, name="r_bc", tag="rbc")
                nsq = smp.tile([1, 1024], F32, name=f"nsq{k}", tag="sm")
                for nh in range(2):
                    for dh in range(2):
                        nc.tensor.matmul(
                            nsq[0:1, 512 * nh:512 * (nh + 1)], t_o1[:],
                            t_sq[:, dh, 512 * nh:512 * (nh + 1)],
                            start=(dh == 0), stop=(dh == 1))
                nln = wk.tile([1, 1024], F32, name="nln", tag="nln")
                nc.scalar.activation(nln[:], nsq[:], AF.Ln)
                t_rk = sb.tile([1, 1024], F32, name=f"t_r{k}")
                t_r_tiles[k] = t_rk
                nc.scalar.activation(t_rk[:], nln[:], AF.Exp, scale=-0.5)
                for nh in range(2):
                    nc.gpsimd.partition_broadcast(
                        r_bc[:, 512 * nh:512 * (nh + 1)],
                        t_rk[0:1, 512 * nh:512 * (nh + 1)])
                for dh in range(2):
                    nc.vector.tensor_tensor(t_on[:, dh, k, :],
                                            t_h[k][:, dh, :], r_bc[:],
                                            ALU.mult)

            def unit(u, a, b, e5slot, mb):
                ps = simp.tile([128, 1024], F32, name="ps", tag="ps")
                for nb in range(2):
                    nc.tensor.matmul(ps[:, 512 * nb:512 * (nb + 1)],
                                     t_on[:, :, a, 128 * mb:128 * (mb + 1)],
                                     t_on[:, :, b, 512 * nb:512 * (nb + 1)],
                                     start=True, stop=True, perf_mode=DR)
                if a == b:
                    nc.vector.tensor_tensor(
                        ps[:], ps[:],
                        t_mask[:, 1024 - 128 * mb:2048 - 128 * mb], ALU.add)
                dest = t_scr[:] if e5slot is None else t_e5[:, mb, e5slot, :]
                nc.scalar.activation(dest, ps[:], AF.Exp, scale=0.15625,
                                     accum_out=rp_st[:, u * 8 + mb:
                                                     u * 8 + mb + 1])

            def stage_full(k):
                for dh in range(2):
                    for h in range(2):
                        head_chain(k, dh, h)
                norm(k)

            def emit_unit(u, mb):
                unit(u, *UNITS[u][:2], UNITS[u][2], mb)

            stage_full(0)
            for k in range(1, 4):
                pu = k - 1
                emit_unit(pu, 0)
                emit_unit(pu, 1)
                head_chain(k, 0, 0)
                emit_unit(pu, 2)
                emit_unit(pu, 3)
                head_chain(k, 0, 1)
                emit_unit(pu, 4)
                emit_unit(pu, 5)
                head_chain(k, 1, 0)
                emit_unit(pu, 6)
                emit_unit(pu, 7)
                head_chain(k, 1, 1)
                norm(k)
            for mb in range(8):
                emit_unit(3, mb)
            for mb in range(8):
                emit_unit(4, mb)

            # pos: bf16 product of t_h slabs 0 and 3, ones-matmul, r-scales
            t_pp = wk.tile([128, 2, 1024], BF16, name="t_pp", tag="sq")
            nc.vector.tensor_tensor(t_pp[:], t_h[0][:], t_h[3][:], ALU.mult)
            pr = smp.tile([1, 1024], F32, name="rawdot", tag="sm")
            for nh in range(2):
                for dh in range(2):
                    nc.tensor.matmul(pr[0:1, 512 * nh:512 * (nh + 1)],
                                     t_o1[:],
                                     t_pp[:, dh, 512 * nh:512 * (nh + 1)],
                                     start=(dh == 0), stop=(dh == 1))
            tmp = wk.tile([1, 1024], F32, name="ptmp", tag="nln")
            nc.vector.tensor_tensor(tmp[:], pr[:], t_r_tiles[0][:], ALU.mult)
            nc.vector.tensor_tensor(ps_st[:], tmp[:], t_r_tiles[3][:],
                                    ALU.mult)

            # colsums from e5 exp tiles (DoubleRow over mb pairs)
            for ci in range(4):
                for nh in range(2):
                    cs = smp.tile([128, 512], F32, name=f"cs{ci}_{nh}",
                                  tag="sm")
                    for jj in range(4):
                        nc.tensor.matmul(
                            cs[:], t_o5[:],
                            t_e5[:, 2 * jj:2 * jj + 2, ci,
                                 512 * nh:512 * (nh + 1)],
                            start=(jj == 0), stop=(jj == 3), perf_mode=DR)
                    nc.vector.tensor_copy(
                        cp_st[0:1, 1024 * ci + 512 * nh:
                              1024 * ci + 512 * (nh + 1)], cs[0:1, :])

            nc.sync.dma_start(o_rp, rp_st[:])
            nc.sync.dma_start(o_cp, cp_st[:])
            nc.sync.dma_start(o_ps, ps_st[:])

    # Keep Exp/Ln selectable only from the single table set that holds both,
    # so the compiler never ping-pongs ACT table loads between exp-only and
    # ln-only sets (1283ns per reload).  Entries stay in place so
    # act_func_set_id indices still match act_info.json.
    import concourse.bacc as bacc_mod
    orig_get = bacc_mod.get_activation_tables

    def _pinned_tables(arch):
        tabs = orig_get(arch)
        AFT = mybir.ActivationFunctionType
        both = [k for k, v in tabs.items() if AFT.Exp in v and AFT.Ln in v]
        if not both:
            return tabs
        keep = both[0]
        out = {}
        for k, v in tabs.items():
            if k == keep:
                out[k] = v
            else:
                out[k] = {f for f in v if f not in (AFT.Exp, AFT.Ln)}
        return out

    bacc_mod.get_activation_tables = _pinned_tables
    try:
        nc.compile()
    finally:
        bacc_mod.get_activation_tables = orig_get
    _CACHE["nc"] = nc
    return nc


def _host_inputs(embedded_data, W, b):
    E4np = ml_dtypes.float8_e4m3
    E5np = ml_dtypes.float8_e5m2
    emb = np.asarray(embedded_data, dtype=np.float32)
    embT8 = np.ascontiguousarray(emb.T).astype(E4np)      # [2048, 8192]
    W8 = (np.asarray(W, dtype=np.float32) * 64.0).astype(E4np)
    b64 = (np.asarray(b, dtype=np.float32) * 64.0).astype(np.float32)
    o1 = np.full((128, 1), 1.0 / 64.0, ml_dtypes.bfloat16)
    o5 = np.ones((128, 256), E5np)
    mask = np.zeros((128, 2048), np.float32)
    mask[np.arange(128), np.arange(128) + 1024] = -1e9
    in_maps = []
    for c in range(8):
        cols = np.concatenate(
            [embT8[:, 1024 * s:1024 * (s + 1)] for s in SLOTS[c]], axis=1)
        in_maps.append({"embT8": np.ascontiguousarray(cols), "W8": W8,
                        "b64": b64, "onesbf": o1, "ones5": o5, "mask": mask})
    return in_maps


def _combine(results):
    neg = np.zeros(8192, np.float64)
    pos = np.zeros(8192, np.float64)
    for c in range(8):
        S = SLOTS[c]
        rp = results[c]["rowpart"].astype(np.float64)     # [128, 40]
        cp = results[c]["colpart"].astype(np.float64).ravel()
        ps = results[c]["possim"].astype(np.float64)
        sl = [np.s_[1024 * s:1024 * (s + 1)] for s in S]
        for u, (astat, _, _) in enumerate(UNITS):
            if u == 3 and c >= 4:
                continue                                   # diff-4 dedup
            dst = 1024 * S[astat]
            for mb in range(8):
                neg[dst + 128 * mb:dst + 128 * (mb + 1)] += rp[:, 8 * u + mb]
        neg[sl[1]] += cp[0:1024]
        neg[sl[2]] += cp[1024:2048]
        if c < 4:
            neg[sl[3]] += cp[2048:3072]
        neg[sl[3]] += cp[3072:4096]
        if c < 4:
            possim = ps.ravel()
            pos[sl[0]] = possim
            pos[sl[3]] = possim
    loss = -np.mean(10.0 * pos - np.log(neg))
    return np.float32(loss)


def run(embedded_data, W, b, trace=False):
    from concourse import bass_utils
    nc = _build()
    in_maps = _host_inputs(embedded_data, W, b)
    res = bass_utils.run_bass_kernel_spmd(nc, in_maps, core_ids=list(range(8)),
                                          trace=trace)
    return _combine(res.results), res


def kernel(embedded_data, W, b):
    loss, _ = run(embedded_data, W, b, trace=False)
    return np.asarray(loss, dtype=np.float32)
